# revision 67
# baseline (speedup 1.0000x reference)
"""FDSCS front-end as two Bass/Tile kernels on 8 Trainium2 NeuronCores.

Kernel A (row-sharded, 8 cores x 48 half-res rows): avg-pool 2x2 -> YCbCr ->
5x5 census on Y emitted as fp8 bit-planes. The Hamming cost volume runs on the
tensor engine as ONE 24-row fp8 matmul per (row, x-block):
  st = 1-2*Rbit in {-1,+1},  mv = Lbit-0.5*[u<W] in {-0.5,+0.5} (0 for u>=W)
  psum = popL - 2*corr + (popR-12)*[u<W] = ham - 12   (exact integers)
Act/DVE/Pool add +12 and cast to uint8; scr stores the raw 2x band as u8
(1152B contiguous rows). The host de-shears the d = u - x band with a
zero-copy as_strided view and applies (ham-MY)/SY during unshard; border rows
and the x >= W-d triangle are constant host fills (no masks exist on device).

Kernel B (disparity-sharded, cyclic d = 8*dp + core): Cb/Cr cost volumes as
SIGNED f8 diffs (planes pre-scaled by the unify constants in kernel A);
engines rotate over {DVE sub->fp8, DVE sub->f16 + Act cast, Pool sub->fp8}.
The host takes |.|, applies the -M/S bias, and fills the right-edge triangle.
"""

import numpy as np

# ---------------------------------------------------------------- constants
N, HF, WF = 2, 384, 1280       # full-res input (per image): (N, 3, HF, WF)
H, W = 192, 640                # half-res
D = 128                        # disparities
NC = 8                         # cores
RPC = H * N // NC              # 48 half-rows per kernel-A core
UW = 768                       # left (u) extent incl. disparity pad
SRW = 1152                     # scr row width: 4 x 256 + 128 (b=4 trimmed)
PITCH = 768                    # staged plane pitch for kernel B
LW = 760                       # kernel-B left-plane width  (W + 15*8)
NDP = 16                       # disparities per kernel-B core (d = 8*dp + c)
NH = N * H                     # 384 half-rows total
GQ = 24                        # staging row-group quantum

MY, SY = 11.08282948, 0.1949711
MU, SU = 0.02175535, 35.91432953
MV, SV = 0.02679042, 26.79782867

OFFSETS = [(0,0),(1,0),(2,0),(3,0),(4,0),(0,1),(1,1),(2,1),(3,1),(4,1),
           (0,2),(1,2),(3,2),(4,2),(0,3),(1,3),(2,3),(3,3),(4,3),
           (0,4),(1,4),(2,4),(3,4),(4,4)]

# partition layout for kernel A: [2 pad][52 L rows][4 pad][52 R rows][2 pad]
P = 112
LB, RB = 2, 58                 # base partitions of the L / R row blocks

_CACHE = {}


def _bass_mods():
    import concourse.bass as bass
    import concourse.tile as tile
    from concourse import bacc, mybir
    return bass, tile, bacc, mybir


# ================================================================ kernel A
def _build_A():
    bass, tile, bacc, mybir = _bass_mods()
    from concourse._compat import with_exitstack
    from contextlib import ExitStack
    dt = mybir.dt
    Alu = mybir.AluOpType
    ActF = mybir.ActivationFunctionType

    nc = bacc.Bacc("TRN2", target_bir_lowering=False, debug=False, num_devices=NC)
    # 112 full-res rows each: pad rows come in as host zeros so every
    # partition of the raw tile is written (L block: partitions 0:56,
    # R block: 56:112; data at half-rows 2:54 of each block).
    rawL = nc.dram_tensor("rawL", (3, P, WF), dt.float32, kind="ExternalInput").ap()
    rawR = nc.dram_tensor("rawR", (3, P, WF), dt.float32, kind="ExternalInput").ap()
    rmt = nc.dram_tensor("rmt", (P, 2), dt.float32, kind="ExternalInput").ap()
    scr = nc.dram_tensor("scr", (RPC, 128, SRW), dt.uint8, kind="ExternalOutput").ap()
    # staged census planes; ExternalOutput DRAM is pre-zeroed, so stgL's
    # cols [W:UW] read back as zeros (the disparity pad of the mv operand).
    stgL = nc.dram_tensor("stgL", (RPC, 24, UW), dt.float8e4, kind="ExternalOutput").ap()
    stgR = nc.dram_tensor("stgR", (RPC, 24, W), dt.float8e4, kind="ExternalOutput").ap()
    outs = {}
    for nm in ("lcb", "lcr", "rcb", "rcr"):
        outs[nm] = nc.dram_tensor(nm, (RPC, W), dt.float16, kind="ExternalOutput").ap()

    @with_exitstack
    def k(ctx: ExitStack, tc):
        vec, gp, act, sy = nc.vector, nc.gpsimd, nc.scalar, nc.sync
        pool = ctx.enter_context(tc.tile_pool(name="prep", bufs=1))

        raw = pool.tile([P, 3 * 2 * WF], dt.float32, name="raw")
        rv = raw[:].rearrange("p (c j x) -> p c j x", c=3, j=2)
        for ci in (2, 1, 0):
            for blk, srct in ((0, rawL), (56, rawR)):
                sy.dma_start(rv[blk:blk + 56, ci],
                             srct.rearrange("c (p j) x -> p c j x", j=2)[:, ci])
        rm = pool.tile([P, 2], dt.float32, name="rm")
        sy.dma_start(rm[:], rmt)

        # pooling: horizontal pair sum, vertical pair sum, x0.25 (exact XLA
        # order); h split per channel so it starts on partially-arrived input
        h = pool.tile([P, 3 * 2 * W], dt.float32, name="h")
        hv = h[:].rearrange("p (c j x) -> p c j x", c=3, j=2)
        for ci in (2, 1):
            vec.tensor_tensor(out=hv[:, ci], in0=rv[:, ci, :, 0::2],
                              in1=rv[:, ci, :, 1::2], op=Alu.add)
        s = pool.tile([P, 3 * W], dt.float32, name="s")
        svw = s[:].rearrange("p (c x) -> p c x", c=3)
        for ci in (2, 1):
            vec.tensor_tensor(out=svw[:, ci], in0=hv[:, ci, 0],
                              in1=hv[:, ci, 1], op=Alu.add)
        # last-arriving channel (c0) in x-halves to shorten the serial chain
        for xh in (0, 1):
            xs = slice(xh * (W // 2), (xh + 1) * (W // 2))
            vec.tensor_tensor(out=hv[:, 0, :, xs],
                              in0=rv[:, 0, :, 2 * xh * (W // 2)::2][:, :, 0:W // 2],
                              in1=rv[:, 0, :, 2 * xh * (W // 2) + 1::2][:, :, 0:W // 2],
                              op=Alu.add)
            vec.tensor_tensor(out=svw[:, 0, xs], in0=hv[:, 0, 0, xs],
                              in1=hv[:, 0, 1, xs], op=Alu.add)
        # Y = (r*.299 + g*.587) + b*.114 on quarter-scaled sums; the 0.25
        # pool scale is a power of two, so folding it into each coefficient
        # is bit-exact vs scaling first.
        sv3 = s[:].rearrange("p (c x) -> p c x", c=3)
        t1 = pool.tile([P, W], dt.float32, name="t1")
        act.activation(t1[:], sv3[:, 0], ActF.Copy, bias=0.0, scale=0.299 * 0.25)
        t2 = pool.tile([P, W], dt.float32, name="t2")
        vec.tensor_scalar(t2[:], sv3[:, 1], 0.587 * 0.25, None, Alu.mult)
        y01 = pool.tile([P, W], dt.float32, name="y01")
        vec.tensor_tensor(out=y01[:], in0=t1[:], in1=t2[:], op=Alu.add)
        t3 = pool.tile([P, W], dt.float32, name="t3")
        act.activation(t3[:], sv3[:, 2], ActF.Copy, bias=0.0, scale=0.114 * 0.25)
        Y = pool.tile([P, W], dt.float32, name="Y")
        vec.tensor_tensor(out=Y[:], in0=y01[:], in1=t3[:], op=Alu.add)

        # Y's pad rows are computed zeros (host-zero raw pads), so the +-2
        # partition shifts pull zeros across image boundaries for free.
        # partition-shifted copies of Y, issued ahead of the cb/cr stores so
        # census unblocks as early as possible
        ysh = {0: Y}
        for dv in (-2, -1, 1, 2):
            t = pool.tile([P, W], dt.float32, name=f"ysh{dv + 2}")
            if dv > 0:
                sy.dma_start(t[0:P - dv], Y[dv:P])
                sy.dma_start(t[P - dv:P], Y[0:dv])   # filler: rows unused
            else:
                sy.dma_start(t[-dv:P], Y[0:P + dv])
                sy.dma_start(t[0:-dv], Y[0:-dv])     # filler: rows unused
            ysh[dv] = t

        # census bits as fp8 planes [P, 24, W]; 2px x-borders stay zero.
        # engine rotation: DVE 663ns / Pool 1261ns per op — Pool takes a share
        # so DVE keeps room for the affine + its norm chunks.
        WI = W - 4
        bits = pool.tile([P, 24 * W], dt.float8e4, name="bits")
        bv = bits[:].rearrange("p (k x) -> p k x", k=24)
        vec.memset(bv[:, :, 0:2], 0.0)
        vec.memset(bv[:, :, W - 2:W], 0.0)
        # census order: v=2 offsets first (no ysh dependency), then k>=14,
        # then k<10. The affine is split so Pool's k-range [10:24) completes
        # early while DVE still runs census; DVE only keeps [0:10).
        bk = bits[:].rearrange("p (k x) -> p k x", k=24)
        order = [10, 11, 12, 13] + list(range(14, 24)) + list(range(10))
        # Pool computes two v=2 bits via sub + (diff>=0)*1 while otherwise
        # idle pre-affine; DVE keeps the remaining 22 (is_ge is DVE-only)
        for k_i in (12, 13):
            u, v = OFFSETS[k_i]
            dfp = pool.tile([P, WI], dt.float32, name=f"dfp{k_i}")
            gp.tensor_tensor(out=dfp[:], in0=ysh[v - 2][:, u:u + WI],
                             in1=Y[:, 2:W - 2], op=Alu.subtract)
            gp.tensor_scalar(bv[:, k_i, 2:W - 2], dfp[:], 0.0, 1.0,
                             Alu.is_ge, Alu.mult)
        for k_i in order:
            if k_i in (12, 13):
                continue
            u, v = OFFSETS[k_i]
            vec.tensor_tensor(out=bv[:, k_i, 2:W - 2],
                              in0=ysh[v - 2][:, u:u + WI],
                              in1=Y[:, 2:W - 2], op=Alu.is_ge)
            if k_i == 15:   # ks 10..15 census done here
                gp.tensor_scalar(bk[:, 10:16], bk[:, 10:16], rm[:, 0:1],
                                 rm[:, 1:2], Alu.mult, Alu.add)
            elif k_i == 23:  # ks 16..23 census done here
                gp.tensor_scalar(bk[:, 16:24], bk[:, 16:24], rm[:, 0:1],
                                 rm[:, 1:2], Alu.mult, Alu.add)
        # L rows: bit - 0.5          R rows: 1 - 2*bit
        vec.tensor_scalar(bk[:, 0:10], bk[:, 0:10], rm[:, 0:1], rm[:, 1:2],
                          Alu.mult, Alu.add)
        # matmul operand planes, k-major: st [24, q, W], mv [24, q, UW];
        # one tile pair per GQ-row group, store/load interleaved so group 0's
        # matmuls start while later groups still stage. Loads ride the Act
        # queue (SP has the stores).
        mplane = ctx.enter_context(tc.tile_pool(name="mplane", bufs=1))
        GRPS = [(0, 4), (4, 12), (16, 32)]  # laddered groups -> earliest start
        for g0, gn in GRPS:
            sy.dma_start(stgL[g0:g0 + gn, :, 0:W], bv[LB + 2 + g0:LB + 2 + g0 + gn])
            sy.dma_start(stgR[g0:g0 + gn], bv[RB + 2 + g0:RB + 2 + g0 + gn])
        stg_tiles = {}
        for g0, gn in GRPS:
            st = mplane.tile([24, gn * W], dt.float8e4, name=f"stp{g0}")
            mv = mplane.tile([24, gn * UW], dt.float8e4, name=f"mvp{g0}")
            sl = bass.AP(stgL.tensor, g0 * 24 * UW, [[UW, 24], [24 * UW, gn], [1, UW]])
            sr = bass.AP(stgR.tensor, g0 * 24 * W, [[W, 24], [24 * W, gn], [1, W]])
            act.dma_start(mv[:].rearrange("p (q u) -> p q u", q=gn), sl)
            act.dma_start(st[:].rearrange("p (q x) -> p q x", q=gn), sr)
            for r in range(g0, g0 + gn):
                stg_tiles[r] = (st, mv, g0, gn)
        # cb = ((b*0.25 - y) * 0.564 + 0.5)/SU ; cr analogous — produced after
        # census so the DVE critical path reaches the affine sooner
        cbd = pool.tile([P, W], dt.float32, name="cbd")
        vec.scalar_tensor_tensor(cbd[:], sv3[:, 2], 0.25, Y[:], Alu.mult,
                                 Alu.subtract)
        cb16 = pool.tile([P, W], dt.float16, name="cb16")
        act.activation(cb16[:], cbd[:], ActF.Copy, bias=0.5 / SU, scale=0.564 / SU)
        crd = pool.tile([P, W], dt.float32, name="crd")
        vec.scalar_tensor_tensor(crd[:], sv3[:, 0], 0.25, Y[:], Alu.mult,
                                 Alu.subtract)
        cr16 = pool.tile([P, W], dt.float16, name="cr16")
        act.activation(cr16[:], crd[:], ActF.Copy, bias=0.5 / SV, scale=0.713 / SV)
        for nm, t, blk in [("lcb", cb16, LB), ("lcr", cr16, LB),
                           ("rcb", cb16, RB), ("rcr", cr16, RB)]:
            sy.dma_start(outs[nm], t[blk + 2:blk + 50, :])

        ppool = ctx.enter_context(tc.tile_pool(name="ps", bufs=2, space="PSUM"))
        pcpool = ctx.enter_context(tc.tile_pool(name="psc", bufs=3, space="PSUM"))
        opool = ctx.enter_context(tc.tile_pool(name="ob", bufs=4))
        # PE pstate warmup: dummy matmuls on the affine-complete bits tile
        # run during the staging window so real matmuls start at full clock
        for wi in range(32):
            wps = pcpool.tile([128, 512], dt.float32, name="psB")
            nc.tensor.matmul(wps[:, 0:256], bits[0:24, 0:128],
                             bits[0:24, 128:384], start=True, stop=True)
        batches = [(r0, 4) for r0 in range(0, RPC - 4, 4)] + [(44, 2), (46, 1), (47, 1)]
        for r0, RB4 in batches:
            o = opool.tile([128, RB4 * SRW], dt.uint8, name="o")
            for ri in range(RB4):
                r = r0 + ri
                stq, mvq, gg0, ggn = stg_tiles[r]
                stv = stq[:].rearrange("p (q x) -> p q x", q=ggn)
                mvv = mvq[:].rearrange("p (q u) -> p q u", q=ggn)
                rq = r - gg0
                psA = ppool.tile([128, 640], dt.float32, name="psA")
                psB = pcpool.tile([128, 512], dt.float32, name="psB")
                for b in range(2):
                    nc.tensor.matmul(
                        psA[:, b * 256:b * 256 + 256],
                        stv[:, rq, b * 128:(b + 1) * 128],
                        mvv[:, rq, b * 128:b * 128 + 256],
                        start=True, stop=True,
                    )
                nc.tensor.matmul(psA[:, 512:640], stv[:, rq, 512:640],
                                 mvv[:, rq, 512:640], start=True, stop=True)
                for b in (2, 3):
                    nc.tensor.matmul(
                        psB[:, (b - 2) * 256:(b - 2) * 256 + 256],
                        stv[:, rq, b * 128:(b + 1) * 128],
                        mvv[:, rq, b * 128:b * 128 + 256],
                        start=True, stop=True,
                    )
                oo = ri * SRW
                # +12 and cast to u8: scr row layout is [b0 b1 b4 | b2 b3];
                # Act takes the 640-chunk 3 rows out of 4 (engine balance:
                # Act-640=718ns Act-512=612, DVE-640=792 DVE-512=658)
                eb, es = (vec, act) if r % 4 == 3 else (act, vec)
                chunks = [(o[:, oo:oo + 640], psA[:], eb),
                          (o[:, oo + 640:oo + 1152], psB[:], es)]
                for dst, ps, eng in chunks:
                    if eng is act:
                        act.activation(dst, ps, ActF.Copy, bias=12.0, scale=1.0)
                    else:
                        eng.tensor_scalar(dst, ps, 1.0, 12.0, Alu.mult, Alu.add)
            steng = act if r0 + RB4 >= RPC else gp
            steng.dma_start(scr[r0:r0 + RB4].rearrange("r p u -> p r u"),
                            o[:].rearrange("p (r u) -> p r u", r=RB4))

    with tile.TileContext(nc) as tc:
        k(tc)
    nc.compile()
    return nc


# ================================================================ kernel B
def _build_B():
    bass, tile, bacc, mybir = _bass_mods()
    from concourse._compat import with_exitstack
    from contextlib import ExitStack
    dt = mybir.dt
    Alu = mybir.AluOpType
    ActF = mybir.ActivationFunctionType

    nc = bacc.Bacc("TRN2", target_bir_lowering=False, debug=False, num_devices=NC)
    ins = {}
    for nm, wid in [("Lcb", LW), ("Lcr", LW), ("Rcb", W), ("Rcr", W)]:
        ins[nm] = nc.dram_tensor(nm, (NH, wid), dt.float16, kind="ExternalInput").ap()
    out = nc.dram_tensor("out", (2, NDP, NH, W), dt.float8e4, kind="ExternalOutput").ap()
    out16 = nc.dram_tensor("out16", (2, NDP, NH, W), dt.float16, kind="ExternalOutput").ap()

    RG = NH // 128  # 3 row groups

    @with_exitstack
    def k(ctx: ExitStack, tc):
        vec, gp, act, sy = nc.vector, nc.gpsimd, nc.scalar, nc.sync

        plane_pool = ctx.enter_context(tc.tile_pool(name="planes", bufs=1))
        planes = {}
        for li, nm in enumerate(("Lcb", "Rcb", "Lcr", "Rcr")):
            wid = LW if nm.startswith("L") else W
            t = plane_pool.tile([128, RG * wid], dt.float16, name=f"pl_{nm}")
            eng = sy if li % 2 == 0 else act
            eng.dma_start(t[:].rearrange("p (g x) -> p g x", g=RG),
                          ins[nm].rearrange("(g p) x -> p g x", p=128))
            planes[nm] = t

        dpool = ctx.enter_context(tc.tile_pool(name="dp", bufs=8))
        fpool = ctx.enter_context(tc.tile_pool(name="fp", bufs=10))

        def Lv(nm, off, wt):
            return planes[nm][:].rearrange("p (g x) -> p g x", g=RG)[:, :, off:off + wt]

        def Rv(nm, wt):
            return planes[nm][:].rearrange("p (g x) -> p g x", g=RG)[:, :, 0:wt]

        # signed diffs only; |.|, bias, and the right-edge triangle are host
        # work during unshard. x >= W-8*dp is never computed (host constant).
        # engine rotation P,A,A,D: Pool-direct 8, DVE-sub+Act-cast 16,
        # DVE-direct 8.
        PAT = ("F", "F", "A", "A") + ("P", "F", "A", "A") * 7
        for dp in range(NDP):
            wt = W - 8 * dp
            for gi, lnm, rnm in ((0, "Lcb", "Rcb"), (1, "Lcr", "Rcr")):
                i = 2 * dp + gi
                kind = PAT[i]
                if kind == "F":
                    # f16 wire for this slot: one DVE op, fatter store
                    c16 = dpool.tile([128, RG * W], dt.float16, name="c16")
                    c16v = c16[:].rearrange("p (g x) -> p g x", g=RG)[:, :, 0:wt]
                    vec.tensor_tensor(out=c16v, in0=Lv(lnm, 8 * dp, wt),
                                      in1=Rv(rnm, wt), op=Alu.subtract)
                    act.dma_start(
                        out16[gi, dp].rearrange("(g p) x -> p g x", p=128)[:, :, 0:wt],
                        c16v)
                    continue
                c8 = fpool.tile([128, RG * W], dt.float8e4, name="c8")
                c8v = c8[:].rearrange("p (g x) -> p g x", g=RG)[:, :, 0:wt]
                if kind == "P":
                    gp.tensor_tensor(out=c8v, in0=Lv(lnm, 8 * dp, wt),
                                     in1=Rv(rnm, wt), op=Alu.subtract)
                else:
                    du = dpool.tile([128, RG * W], dt.float16, name="du")
                    duv = du[:].rearrange("p (g x) -> p g x", g=RG)[:, :, 0:wt]
                    vec.tensor_tensor(out=duv, in0=Lv(lnm, 8 * dp, wt),
                                      in1=Rv(rnm, wt), op=Alu.subtract)
                    act.activation(c8v, duv, ActF.Copy, bias=0.0, scale=1.0)
                sy.dma_start(
                    out[gi, dp].rearrange("(g p) x -> p g x", p=128)[:, :, 0:wt],
                    c8v)

    with tile.TileContext(nc) as tc:
        k(tc)
    nc.compile()
    return nc


# ================================================================ host
def _run(nc, in_maps):
    from concourse.bass_utils import run_bass_kernel_spmd
    return run_bass_kernel_spmd(nc, in_maps, core_ids=list(range(NC)))


def kernel(left, right):
    left = np.asarray(left, dtype=np.float32)
    right = np.asarray(right, dtype=np.float32)

    if "A" not in _CACHE:
        _CACHE["A"] = _build_A()
    if "B" not in _CACHE:
        _CACHE["B"] = _build_B()

    # ---------------- kernel A launch (row-sharded)
    rmv = np.zeros((P, 2), np.float32)
    rmv[LB:LB + 52] = (1.0, -0.5)      # L rows: bit - 0.5
    rmv[RB:RB + 52] = (-2.0, 1.0)      # R rows: 1 - 2*bit
    in_mapsA = []
    for c in range(NC):
        n, r0 = c // 4, 48 * (c % 4)
        lo, hi = 2 * r0 - 4, 2 * (r0 + RPC) + 4
        slL = np.zeros((3, P, WF), np.float32)
        slR = np.zeros((3, P, WF), np.float32)
        clo, chi = max(lo, 0), min(hi, HF)
        # data occupies full-res rows [4:108] (pad rows 0:4 / 108:112 stay 0)
        slL[:, 4 + clo - lo:108 - (hi - chi)] = left[n, :, clo:chi]
        slR[:, 4 + clo - lo:108 - (hi - chi)] = right[n, :, clo:chi]
        in_mapsA.append({"rawL": slL, "rawR": slR, "rmt": rmv})
    resA = _run(_CACHE["A"], in_mapsA)

    # ---------------- assemble staged canvases for kernel B
    canv = {nm: np.zeros((NH, PITCH), np.float16)
            for nm in ("lcb", "lcr", "rcb", "rcr")}
    for c in range(NC):
        for nm in canv:
            canv[nm][48 * c:48 * (c + 1), :W] = resA.results[c][nm]

    # ---------------- kernel B launch (disparity-sharded)
    in_mapsB = []
    for c in range(NC):
        m = {
            "Lcb": np.ascontiguousarray(canv["lcb"][:, c:c + LW]),
            "Lcr": np.ascontiguousarray(canv["lcr"][:, c:c + LW]),
            "Rcb": np.ascontiguousarray(canv["rcb"][:, :W]),
            "Rcr": np.ascontiguousarray(canv["rcr"][:, :W]),
        }
        in_mapsB.append(m)
    resB = _run(_CACHE["B"], in_mapsB)

    # ---------------- assemble output
    outf = np.empty((N, 3 * D, H, W), np.float32)
    # y-group: de-shear the u8 band scratch (d = u - x) during unshard.
    # 256-byte tail pad keeps the b=4 as_strided view in-bounds.
    flat = np.zeros(NH * 128 * SRW + 256, np.uint8)
    scr = flat[:NH * 128 * SRW].reshape(NH, 128, SRW)
    for c in range(NC):
        scr[48 * c:48 * (c + 1)] = resA.results[c]["scr"]
    s0 = 128 * SRW
    v01 = np.lib.stride_tricks.as_strided(
        scr, shape=(D, NH, 2, 128), strides=(1, s0, 256, SRW + 1))
    v23 = np.lib.stride_tricks.as_strided(
        scr[:, :, 640:], shape=(D, NH, 2, 128), strides=(1, s0, 256, SRW + 1))
    v4 = np.lib.stride_tricks.as_strided(
        scr[:, :, 512:], shape=(D, NH, 128), strides=(1, s0, SRW + 1))
    yf = np.empty((D, NH, W), np.float32)
    yf[:, :, 0:256] = v01.reshape(D, NH, 256)
    yf[:, :, 256:512] = v23.reshape(D, NH, 256)
    yf[:, :, 512:W] = v4
    yf -= np.float32(MY)
    yf *= np.float32(1.0 / SY)
    for n in range(N):
        outf[n, 0:D] = yf[:, n * H:(n + 1) * H]
    # u/v groups from kernel B: |.| + bias during unshard
    PATB = ("F", "F", "A", "A") + ("P", "F", "A", "A") * 7
    for c in range(NC):
        o8 = resB.results[c]["out"]
        o16 = resB.results[c]["out16"]
        for gi, bias in ((0, MU / SU), (1, MV / SV)):
            for dp in range(NDP):
                src_ = o16 if PATB[2 * dp + gi] == "F" else o8
                v = np.abs(src_[gi, dp].astype(np.float32)).reshape(N, H, W)
                outf[:, (1 + gi) * D + 8 * dp + c] = v - np.float32(bias)
    # constant fills: x >= W-d triangle (all groups) and census border rows (y)
    for gi, cst in ((0, -MY / SY), (1, -MU / SU), (2, -MV / SV)):
        for d in range(1, D):
            outf[:, gi * D + d, :, W - d:] = np.float32(cst)
    outf[:, 0:D, (0, 1, H - 2, H - 1), :] = np.float32(-MY / SY)
    return outf
